# revision 21
# baseline (speedup 1.0000x reference)
"""GCN (5x GCNConv + global_mean_pool + 2-layer MLP) on 8 Trainium2 cores.

Strategy (node-partition, per the sharding hint):
  - Nodes sharded across 8 cores (12500 each, padded to 12800 = 25x512).
  - dinv (deg^-1/2) and per-graph 1/count computed on host (cheap) and
    shipped as inputs - no device degree pass.
  - Per layer: GEMM per shard (feat-major fp16), scale by dinv, transpose
    (TensorE), AllGather the scaled features in 4 quarter chunks
    (25600-row tables -> int16-indexable), then per chunk j (j-outer):
    dma_gather source rows per edge and scatter-add into dst supertiles
    via one-hot matmuls accumulated in PSUM (dinv dst-scale folded into
    the one-hot), with per-(st,j) PSUM partials accumulated into an SBUF
    fp16 accumulator so AG chunk j+1 overlaps chunk j's SpMM.
  - Epilogue per supertile fused into the last chunk: self-loop term +
    bias + relu; per-graph mean pooling via one-hot (node->graph)
    matmuls; partial sums AllReduced; small MLP replicated; core 0 out.
Compute dtypes: fp16 storage / fp32 accumulation (PSUM), MLP in fp32.
"""

import numpy as np

NC = 8
_G_DEFAULT = 512
FP16 = np.float16


def _ceil_to(a, m):
    return -(-a // m) * m


def _preprocess(x, edge_index, batch, n_graphs):
    """Build per-core edge streams and static structure."""
    N, D = x.shape
    assert N % NC == 0
    SH = N // NC                      # real rows per core
    SHP = _ceil_to(SH, 512)           # padded rows per core
    QT = SHP // 4                     # quarter (AllGather chunk per core)
    NT = SHP // 128                   # node tiles per core
    NST = SHP // 512                  # supertiles per core
    TBL = NC * QT                     # rows per gathered chunk table
    assert TBL < 32768, "int16 gather index overflow"
    G = n_graphs

    row = np.asarray(edge_index[0], dtype=np.int64)
    col = np.asarray(edge_index[1], dtype=np.int64)
    # self-loops are NOT materialized as edges; their contribution is added
    # during the epilogue (dinv^2*u term) and deg gets +1 on host.

    # host-side degree / dinv (symmetric GCN norm incl. self loop)
    deg = np.bincount(col, minlength=N).astype(np.float64) + 1.0
    dinv = (1.0 / np.sqrt(deg)).astype(np.float32)          # [N]

    kd = col // SH                    # destination core
    ld = col - kd * SH                # local dst row
    ks = row // SH                    # source core
    rr = row - ks * SH
    jq = rr // QT                     # source quarter (0..3)
    idx16 = (ks * QT + (rr - jq * QT)).astype(np.int64)
    tile = ld // 128
    stile = tile // 4

    # per-core sorted streams: (supertile, j, tile, ld)
    per_core = []
    for k in range(NC):
        m = kd == k
        o = np.lexsort((ld[m], tile[m], jq[m], stile[m]))
        per_core.append({
            "tile": tile[m][o], "j": jq[m][o],
            "idx16": idx16[m][o], "ld": ld[m][o],
        })

    # static cell sizes: cell = (tile, j); cross-core max (SPMD: identical
    # window list on all cores); packed tight (no 16-rounding).
    ncell = NT * 4
    S = np.zeros(ncell, dtype=np.int64)
    for k in range(NC):
        ck = per_core[k]["tile"] * 4 + per_core[k]["j"]
        cnt = np.bincount(ck, minlength=ncell)
        S = np.maximum(S, cnt)
    # every (tile, j) cell gets >=1 slot so every (st, j) PSUM bracket and
    # every epilogue exists (pad slots carry scal=-1000 -> zero one-hot)
    S = np.maximum(S, 1)

    # slot layout: groups (st, j) padded to 128-multiples; cells within a
    # group contiguous, tight-packed.
    cell_off = np.zeros(ncell, dtype=np.int64)   # slot offset by cell id
    off = 0
    for st in range(NST):
        for j in range(4):
            for a in range(4):
                c = (4 * st + a) * 4 + j
                cell_off[c] = off
                off += S[c]
            off = _ceil_to(off, 128)   # pad group end to 128
    TOT = off                          # total slots per core

    # group (st, j) sizes/offsets for gathers
    groups = []      # (st, j, slot_off, slots, padded_slots)
    for st in range(NST):
        for j in range(4):
            c0 = (4 * st) * 4 + j
            goff = cell_off[c0]
            gsz = int(sum(S[(4 * st + a) * 4 + j] for a in range(4)))
            gpad = _ceil_to(gsz, 128)
            groups.append((st, j, int(goff), gsz, gpad))

    # fill per-core padded streams
    idx_slots = np.zeros((NC, TOT), dtype=np.int16)
    scal_slots = np.full((NC, TOT), -1000.0, dtype=np.float32)
    for k in range(NC):
        pk = per_core[k]
        ck = pk["tile"] * 4 + pk["j"]
        arange = np.arange(len(ck))
        if len(ck):
            starts_pos = np.concatenate(
                [[0], np.flatnonzero(np.diff(ck) != 0) + 1])
            first_occ = np.zeros(ncell, dtype=np.int64)
            first_occ[ck[starts_pos]] = starts_pos
            within = arange - first_occ[ck]
        else:
            within = arange
        slot = cell_off[ck] + within
        idx_slots[k, slot] = pk["idx16"].astype(np.int16)
        scal_slots[k, slot] = pk["ld"].astype(np.float32)

    # wrapped idx layout per gather group: [16, S/16] tiled to [128, S/16]
    IDXCOLS = TOT // 16
    idx_stream = np.zeros((NC, 128, IDXCOLS), dtype=np.int16)
    gcol_off = {}
    coff = 0
    for (st, j, goff, gsz, gpad) in groups:
        gcol_off[(st, j)] = coff
        if gpad == 0:
            continue
        blk = idx_slots[:, goff:goff + gpad].reshape(NC, gpad // 16, 16)
        blk = np.transpose(blk, (0, 2, 1))        # [NC, 16, S/16]
        idx_stream[:, :, coff:coff + gpad // 16] = np.tile(blk, (1, 8, 1))
        coff += gpad // 16

    # windows: one scatter-matmul per (128-slot chunk x intersecting cell).
    ld_slots = scal_slots
    win_cols = []
    chunks = []
    for (st, j, goff, gsz, gpad) in groups:
        for a in range(4):
            c = (4 * st + a) * 4 + j
            if S[c] == 0:
                continue
            c0, c1 = cell_off[c], cell_off[c] + int(S[c])
            ch_lo, ch_hi = c0 // 128, (c1 - 1) // 128
            for ci in range(ch_lo, ch_hi + 1):
                slot0 = ci * 128
                colv = ld_slots[:, slot0:slot0 + 128] - 128.0 * (4 * st + a)
                win_cols.append(colv.astype(np.float32))
                chunks.append(dict(
                    st=st, j=j, a=a,
                    tok_col=int(slot0 - goff) // 128,
                    scal_col=len(win_cols) - 1,
                    base=a * 128,
                ))
    NWIN = len(win_cols)
    scal_stream = np.stack(win_cols, axis=2)  # [NC, 128, NWIN]

    # pooling: batch scalars per core per node tile [128, NT]
    batch = np.asarray(batch, dtype=np.int64)
    batch_scal = np.full((NC, 128, NT), -1000.0, dtype=np.float32)
    for k in range(NC):
        bs = batch[k * SH:(k + 1) * SH].astype(np.float32)
        pad = np.full(SHP - SH, -1000.0, dtype=np.float32)
        bp = np.concatenate([bs, pad]).reshape(NT, 128).T
        batch_scal[k] = bp

    # per-graph 1/count (host): replicated [128, 512]
    cnt = np.bincount(batch, minlength=G).astype(np.float32)
    recip = (1.0 / np.maximum(cnt, 1.0)).astype(np.float32)
    recip_rep = np.tile(recip[None, :], (128, 1))            # [128, G]

    # per-core dinv replicated [128, SHP] fp16
    dinv_rep = np.zeros((NC, 128, SHP), dtype=FP16)
    for k in range(NC):
        dv = np.zeros(SHP, dtype=np.float32)
        dv[:SH] = dinv[k * SH:(k + 1) * SH]
        dinv_rep[k] = np.tile(dv[None, :].astype(FP16), (128, 1))

    # AG-in DMA segments per supertile: (tile_a0, ntiles, j, rowoff)
    ag_segs = []
    for st in range(NST):
        segs = []
        a = 0
        while a < 4:
            base = 512 * st + 128 * a
            j = base // QT
            r = base - j * QT
            n = 1
            while a + n < 4 and (base + 128 * n) // QT == j:
                n += 1
            segs.append((a, n, j, r))
            a += n
        ag_segs.append(segs)

    meta = dict(
        N=N, D=D, SH=SH, SHP=SHP, QT=QT, NT=NT, NST=NST, TBL=TBL, G=G,
        TOT=TOT, NWIN=NWIN, IDXCOLS=IDXCOLS,
        groups=groups, gcol_off=gcol_off, chunks=chunks, ag_segs=ag_segs,
        idx_stream=idx_stream, scal_stream=scal_stream,
        batch_scal=batch_scal, dinv_rep=dinv_rep, recip_rep=recip_rep,
    )
    return meta


def _build(meta):
    """Construct the Bass module (SPMD; identical program on 8 cores)."""
    import concourse.mybir as mybir
    import concourse.bacc as bacc
    import concourse.tile as tile

    f32 = mybir.dt.float32
    fp16 = mybir.dt.float16
    i16 = mybir.dt.int16

    SHP, QT, NT, NST, TBL, G = (meta["SHP"], meta["QT"], meta["NT"],
                                meta["NST"], meta["TBL"], meta["G"])
    NWIN, IDXCOLS = meta["NWIN"], meta["IDXCOLS"]
    groups, gcol_off, chunks, ag_segs = (meta["groups"], meta["gcol_off"],
                                         meta["chunks"], meta["ag_segs"])
    MAXGCOL = max((g[4] // 128 for g in groups), default=1)

    nc = bacc.Bacc("TRN2", target_bir_lowering=False, debug=False,
                   enable_asserts=False, num_devices=NC)

    # ---- I/O ----
    xT_in = nc.dram_tensor("xT_in", [128, SHP], f32, kind="ExternalInput")
    idx_in = nc.dram_tensor("idx_in", [128, IDXCOLS], i16, kind="ExternalInput")
    scal_in = nc.dram_tensor("scal_in", [128, NWIN], f32, kind="ExternalInput")
    bscal_in = nc.dram_tensor("bscal_in", [128, NT], f32, kind="ExternalInput")
    dinv_in = nc.dram_tensor("dinv_in", [128, SHP], fp16, kind="ExternalInput")
    recip_in = nc.dram_tensor("recip_in", [128, G], f32, kind="ExternalInput")
    w_in = nc.dram_tensor("w_in", [5 * 128, 128], fp16, kind="ExternalInput")
    ball_in = nc.dram_tensor("ball_in", [128, 5], f32, kind="ExternalInput")
    iota128_in = nc.dram_tensor("iota128_in", [128, 128], fp16, kind="ExternalInput")
    iotag_in = nc.dram_tensor("iotag_in", [128, G], fp16, kind="ExternalInput")
    ident_in = nc.dram_tensor("ident_in", [128, 128], fp16, kind="ExternalInput")
    wl1_in = nc.dram_tensor("wl1_in", [640, 640], f32, kind="ExternalInput")
    bl1_in = nc.dram_tensor("bl1_in", [128, 5], f32, kind="ExternalInput")
    wl2_in = nc.dram_tensor("wl2_in", [128, 5], f32, kind="ExternalInput")
    bl2_in = nc.dram_tensor("bl2_in", [1, 1], f32, kind="ExternalInput")
    out_ext = nc.dram_tensor("out", [G], f32, kind="ExternalOutput")

    # ---- internal DRAM (collectives) ----
    ag_ins, ag_outs = [], []
    for l in range(5):
        ag_ins.append([nc.dram_tensor(f"agi_{l}_{j}", [QT, 128], fp16,
                                      kind="Internal") for j in range(4)])
        ag_outs.append([nc.dram_tensor(f"ago_{l}_{j}", [TBL, 128], fp16,
                                       kind="Internal", addr_space="Shared")
                        for j in range(4)])
    ar_in = nc.dram_tensor("ar_in", [640, 512], f32, kind="Internal")
    ar_out = nc.dram_tensor("ar_out", [640, 512], f32, kind="Internal",
                            addr_space="Shared")

    AOT = mybir.AluOpType
    AFT = mybir.ActivationFunctionType

    ch_by_stj = {}
    for ch in chunks:
        ch_by_stj.setdefault((ch["st"], ch["j"]), []).append(ch)
    # S>=1 forcing guarantees every (st, j) bracket and epilogue exists
    assert all(ch_by_stj.get((st, j)) for st in range(NST) for j in range(4))

    with tile.TileContext(nc) as tc:
        with tc.tile_pool(name="const", bufs=1) as cpool, \
             tc.tile_pool(name="stream", bufs=1) as spool, \
             tc.tile_pool(name="big", bufs=1) as bpool, \
             tc.tile_pool(name="work", bufs=2) as wpool, \
             tc.tile_pool(name="tokp", bufs=3) as tokpool, \
             tc.tile_pool(name="idxp", bufs=2) as ipool, \
             tc.tile_pool(name="mp", bufs=4) as mpool, \
             tc.tile_pool(name="psA", bufs=2, space="PSUM") as psA, \
             tc.tile_pool(name="psB", bufs=2, space="PSUM") as psB, \
             tc.tile_pool(name="psP", bufs=1, space="PSUM") as psP:

            # ---- constants ----
            iota128 = cpool.tile([128, 128], fp16)
            nc.sync.dma_start(iota128[:], iota128_in.ap())
            iotag = cpool.tile([128, G], fp16)
            nc.sync.dma_start(iotag[:], iotag_in.ap())
            ident = cpool.tile([128, 128], fp16)
            nc.sync.dma_start(ident[:], ident_in.ap())
            w_sb = cpool.tile([128, 5, 128], fp16)
            nc.sync.dma_start(w_sb[:], w_in.ap().rearrange("(a p) b -> p a b", p=128))
            ball = cpool.tile([128, 5], f32)
            nc.sync.dma_start(ball[:], ball_in.ap())
            bscal = cpool.tile([128, NT], f32)
            nc.sync.dma_start(bscal[:], bscal_in.ap())
            dinv_rep = bpool.tile([128, SHP], fp16)
            nc.sync.dma_start(dinv_rep[:], dinv_in.ap())

            scal_sb = spool.tile([128, NWIN], f32)
            nc.sync.dma_start(scal_sb[:], scal_in.ap())

            # y ping-pong buffers (feat-major, fp16)
            yT = [bpool.tile([128, SHP], fp16, name=f"yT{i}", tag=f"yT{i}")
                  for i in range(2)]
            nc.gpsimd.dma_start(yT[0][:], xT_in.ap())   # cast f32->fp16

            # SBUF accumulator for chunk partial sums (chunk 0 copies over
            # whatever is there, chunks 1-3 add)
            acc = bpool.tile([128, SHP], fp16, name="acc", tag="acc")

            def gemm_st(l, st):
                """GEMM + dinv-scale + transpose + AG-input DMAs for one
                supertile of layer l; leaves ut=dinv*u in ycur's slice."""
                ycur_l = yT[l % 2]
                s0 = 512 * st
                ups = psA.tile([128, 512], f32, tag="mm")
                nc.tensor.matmul(ups[:], w_sb[:, l, :],
                                 ycur_l[:, s0:s0 + 512], start=True, stop=True)
                ut = ycur_l[:, s0:s0 + 512]   # reuse consumed input buffer
                nc.vector.tensor_tensor(ut, ups[:],
                                        dinv_rep[:, s0:s0 + 512], AOT.mult)
                trp = psB.tile([128, 512], fp16, tag="tr")
                for a in range(4):
                    nc.tensor.transpose(trp[:, 128 * a:128 * a + 128],
                                        ut[:, 128 * a:128 * a + 128], ident[:])
                agst = wpool.tile([128, 4, 128], fp16, tag="agst")
                nc.vector.tensor_copy(
                    agst[:].rearrange("p a b -> p (a b)"), trp[:])
                for (a0, ntil, j, roff) in ag_segs[st]:
                    nc.sync.dma_start(
                        ag_ins[l][j].ap()[roff:roff + 128 * ntil, :]
                        .rearrange("(a p) b -> p a b", p=128),
                        agst[:, a0:a0 + ntil, :])

            def fire_ag(l, jj):
                nc.gpsimd.collective_compute(
                    "AllGather", AOT.bypass,
                    replica_groups=[list(range(NC))],
                    ins=[ag_ins[l][jj].ap().opt()],
                    outs=[ag_outs[l][jj].ap().opt()])

            # AG chunk jj is complete once this supertile's gemm is done
            ag_bound = {((jj + 1) * QT - 1) // 512: jj for jj in range(4)}

            pool_ps = None
            # layer 0 prologue: gemm + AGs (later layers interleave into the
            # previous layer's epilogue loop)
            scope_g = nc.named_scope("L0_gemm")
            scope_g.__enter__()
            for st in range(NST):
                gemm_st(0, st)
                if st in ag_bound:
                    fire_ag(0, ag_bound[st])
            scope_g.__exit__(None, None, None)

            for l in range(5):
                ycur, ynext = yT[l % 2], yT[(l + 1) % 2]
                # ---- SpMM, chunk-outer (j, then st) ----
                scope_s = nc.named_scope(f"L{l}_spmm")
                scope_s.__enter__()
                for j in range(4):
                    for st in range(NST):
                        cl = ch_by_stj.get((st, j))
                        if not cl:
                            continue
                        gidx = 4 * st + j
                        (gst, gj, goff, gsz, gpad) = groups[gidx]
                        assert gst == st and gj == j
                        if gpad == 0:
                            continue
                        tok = tokpool.tile([128, MAXGCOL, 128], fp16, tag="tok")
                        co = gcol_off[(st, j)]
                        idxt = ipool.tile([128, 8 * MAXGCOL], i16, tag="idx")
                        nc.sync.dma_start(idxt[:, :gpad // 16],
                                          idx_in.ap()[:, co:co + gpad // 16])
                        nc.gpsimd.dma_gather(
                            tok[:, :gpad // 128, :], ag_outs[l][j].ap(),
                            idxt[:, :gpad // 16],
                            num_idxs=gpad, num_idxs_reg=gpad, elem_size=128,
                            single_packet=False,
                        )
                        zps = psA.tile([128, 512], f32, tag="mm")
                        s0 = 512 * st
                        for i, ch in enumerate(cl):
                            m = mpool.tile([128, 128], fp16, tag="M")
                            d0 = s0 + ch["base"]
                            nc.vector.scalar_tensor_tensor(
                                m[:], iota128[:],
                                scal_sb[:, ch["scal_col"]:ch["scal_col"] + 1],
                                dinv_rep[:, d0:d0 + 128],
                                AOT.is_equal, AOT.mult)
                            nc.tensor.matmul(
                                zps[:, ch["base"]:ch["base"] + 128],
                                tok[:, ch["tok_col"], :], m[:],
                                start=(i == 0), stop=(i == len(cl) - 1))
                        # accumulate chunk partial into SBUF acc
                        if j == 0:
                            nc.vector.tensor_copy(acc[:, s0:s0 + 512], zps[:])
                        else:
                            nc.vector.tensor_tensor(acc[:, s0:s0 + 512],
                                                    zps[:],
                                                    acc[:, s0:s0 + 512],
                                                    AOT.add)
                        if j < 3:
                            continue
                        # ---- epilogue (after last chunk of this st) ----
                        # y = relu(z + dinv^2*u + b); ut = dinv*u is in ycur
                        selft = wpool.tile([128, 512], f32, tag="selft")
                        nc.vector.tensor_tensor(selft[:], ycur[:, s0:s0 + 512],
                                                dinv_rep[:, s0:s0 + 512],
                                                AOT.mult)
                        nc.vector.tensor_tensor(selft[:], acc[:, s0:s0 + 512],
                                                selft[:], AOT.add)
                        nc.scalar.activation(ynext[:, s0:s0 + 512], selft[:],
                                             AFT.Relu, bias=ball[:, l:l + 1])
                        # ---- pooling (node->graph one-hot matmuls) ----
                        trp2 = psB.tile([128, 512], fp16, tag="tr")
                        for a in range(4):
                            nc.tensor.transpose(
                                trp2[:, 128 * a:128 * a + 128],
                                ynext[:, s0 + 128 * a:s0 + 128 * (a + 1)],
                                ident[:])
                        ynm = wpool.tile([128, 4, 128], fp16, tag="ynm")
                        nc.vector.tensor_copy(
                            ynm[:].rearrange("p a b -> p (a b)"), trp2[:])
                        if st == 0:
                            pool_ps = psP.tile([128, 512], f32, tag="pool")
                        for a in range(4):
                            t = 4 * st + a
                            mp = mpool.tile([128, G], fp16, tag="Mp")
                            nc.vector.tensor_scalar(
                                mp[:], iotag[:], bscal[:, t:t + 1], None,
                                AOT.is_equal)
                            nc.tensor.matmul(
                                pool_ps[:, :G], ynm[:, a, :], mp[:],
                                start=(st == 0 and a == 0),
                                stop=(st == NST - 1 and a == 3))
                        # ---- next layer's GEMM for this supertile + AG ----
                        if l < 4:
                            gemm_st(l + 1, st)
                            if st in ag_bound:
                                fire_ag(l + 1, ag_bound[st])
                scope_s.__exit__(None, None, None)
                arst = wpool.tile([128, 512], f32, tag="arst")
                nc.vector.tensor_copy(arst[:, :G], pool_ps[:, :G])
                if G < 512:
                    nc.vector.memset(arst[:, G:], 0.0)
                nc.sync.dma_start(ar_in.ap()[128 * l:128 * (l + 1), :], arst[:])
                # per-layer AllReduce slice: layers 0-3 hide under gathers
                nc.gpsimd.collective_compute(
                    "AllReduce", AOT.add, replica_groups=[list(range(NC))],
                    ins=[ar_in.ap()[128 * l:128 * (l + 1), :].opt()],
                    outs=[ar_out.ap()[128 * l:128 * (l + 1), :].opt()])

            # ---- MLP (replicated, fp32) ----
            wl1 = bpool.tile([128, 5, 640], f32)
            nc.sync.dma_start(wl1[:],
                              wl1_in.ap().rearrange("(a p) b -> p a b", p=128))
            wl2 = cpool.tile([128, 5], f32)
            nc.sync.dma_start(wl2[:], wl2_in.ap())
            bl1 = cpool.tile([128, 5], f32)
            nc.sync.dma_start(bl1[:], bl1_in.ap())
            bl2 = cpool.tile([1, 1], f32)
            nc.sync.dma_start(bl2[:], bl2_in.ap())
            rrep = cpool.tile([128, G], f32)
            nc.sync.dma_start(rrep[:], recip_in.ap())

            pm = [wpool.tile([128, 512], f32, tag=f"pm{t}", bufs=1,
                             name=f"pm{t}") for t in range(5)]
            for t in range(5):
                pt = wpool.tile([128, 512], f32, tag="pt")
                nc.sync.dma_start(pt[:], ar_out.ap()[128 * t:128 * (t + 1), :])
                nc.vector.tensor_tensor(pm[t][:, :G], pt[:, :G], rrep[:],
                                        AOT.mult)
                if G < 512:
                    nc.vector.memset(pm[t][:, G:], 0.0)
            yps = psP.tile([1, 512], f32, tag="yf")
            for o in range(5):
                hps = psA.tile([128, 512], f32, tag="mm")
                for i in range(5):
                    nc.tensor.matmul(hps[:], wl1[:, i, 128 * o:128 * (o + 1)],
                                     pm[i][:], start=(i == 0), stop=(i == 4))
                hso = wpool.tile([128, 512], f32, tag="hs")
                nc.scalar.activation(hso[:], hps[:], AFT.Relu,
                                     bias=bl1[:, o:o + 1])
                nc.tensor.matmul(yps[:], wl2[:, o:o + 1], hso[:],
                                 start=(o == 0), stop=(o == 4))
            ysb = wpool.tile([1, 512], f32, tag="ysb")
            nc.scalar.activation(ysb[:], yps[:], AFT.Identity, bias=bl2[:, 0:1])
            nc.sync.dma_start(out_ext.ap().rearrange("(a b) -> a b", a=1),
                              ysb[:, :G])

    nc.compile()
    return nc


def _make_in_maps(meta, x, W_list, b_list, Wl1, bl1, Wl2, bl2):
    N, D, SH, SHP, NT, G = (meta["N"], meta["D"], meta["SH"], meta["SHP"],
                            meta["NT"], meta["G"])
    iota128 = np.tile(np.arange(128), (128, 1)).astype(FP16)
    iotag = np.tile(np.arange(G), (128, 1)).astype(FP16)
    ident = np.eye(128).astype(FP16)
    w_stack = np.concatenate([w.astype(FP16) for w in W_list], axis=0)  # [640,128]
    ball = np.stack([b.astype(np.float32) for b in b_list], axis=1)     # [128,5]
    bl1m = np.asarray(bl1, np.float32).reshape(5, 128).T                # [128,5]
    wl2m = np.asarray(Wl2, np.float32).reshape(5, 128).T                # [128,5]
    wl1m = np.asarray(Wl1, np.float32)
    bl2m = np.asarray(bl2, np.float32).reshape(1, 1)

    in_maps = []
    for k in range(NC):
        xs = np.asarray(x[k * SH:(k + 1) * SH], np.float32)
        xT = np.zeros((128, SHP), np.float32)
        xT[:, :SH] = xs.T
        in_maps.append(dict(
            xT_in=xT, idx_in=meta["idx_stream"][k],
            scal_in=meta["scal_stream"][k], bscal_in=meta["batch_scal"][k],
            dinv_in=meta["dinv_rep"][k], recip_in=meta["recip_rep"],
            w_in=w_stack, ball_in=ball, iota128_in=iota128, iotag_in=iotag,
            ident_in=ident,
            wl1_in=wl1m, bl1_in=bl1m, wl2_in=wl2m, bl2_in=bl2m,
        ))
    return in_maps


_LAST_RESULT = {}


def kernel(x, edge_index, batch, W1, b1, W2, b2, W3, b3, W4, b4,
           Wl1, bl1, Wl2, bl2, n_graphs=_G_DEFAULT, trace=False):
    from concourse import bass_utils

    x = np.asarray(x)
    meta = _preprocess(x, np.asarray(edge_index), np.asarray(batch), n_graphs)
    nc = _build(meta)
    in_maps = _make_in_maps(meta, x, [W1, W2, W3, W4, W4],
                            [b1, b2, b3, b4, b4], Wl1, bl1, Wl2, bl2)
    res = bass_utils.run_bass_kernel_spmd(
        nc, in_maps, core_ids=list(range(NC)), trace=trace)
    _LAST_RESULT["res"] = res
    return res.results[0]["out"].astype(np.float32)


# revision 28
# speedup vs baseline: 1.2168x; 1.2168x over previous
"""GCN (5x GCNConv + global_mean_pool + 2-layer MLP) on 8 Trainium2 cores.

Strategy (node-partition, per the sharding hint):
  - Nodes sharded across 8 cores (12500 each, padded to 12800 = 25x512).
  - dinv (deg^-1/2) and per-graph 1/count computed on host (cheap) and
    shipped as inputs - no device degree pass.
  - Per layer: GEMM per shard (feat-major fp16), scale by dinv, transpose
    (TensorE), AllGather the scaled features in 4 quarter chunks
    (25600-row tables -> int16-indexable), then per chunk j (j-outer):
    dma_gather source rows per edge and scatter-add into dst supertiles
    via one-hot matmuls accumulated in PSUM (dinv dst-scale folded into
    the one-hot), with per-(st,j) PSUM partials accumulated into an SBUF
    fp16 accumulator so AG chunk j+1 overlaps chunk j's SpMM.
  - Epilogue per supertile fused into the last chunk: self-loop term +
    bias + relu; per-graph mean pooling via one-hot (node->graph)
    matmuls; partial sums AllReduced; small MLP replicated; core 0 out.
Compute dtypes: fp16 storage / fp32 accumulation (PSUM), MLP in fp32.
"""

import numpy as np

NC = 8
_G_DEFAULT = 512
FP16 = np.float16


def _ceil_to(a, m):
    return -(-a // m) * m


def _preprocess(x, edge_index, batch, n_graphs):
    """Build per-core edge streams and static structure."""
    N, D = x.shape
    assert N % NC == 0
    SH = N // NC                      # real rows per core
    SHP = _ceil_to(SH, 512)           # padded rows per core
    QT = SHP // 4                     # quarter (AllGather chunk per core)
    NT = SHP // 128                   # node tiles per core
    NST = SHP // 512                  # supertiles per core
    TBL = NC * QT                     # rows per gathered chunk table
    assert TBL < 32768, "int16 gather index overflow"
    G = n_graphs

    row = np.asarray(edge_index[0], dtype=np.int64)
    col = np.asarray(edge_index[1], dtype=np.int64)
    batch = np.asarray(batch, dtype=np.int64)
    # self-loops are NOT materialized as edges; their contribution is added
    # during the epilogue (dinv^2*u term) and deg gets +1 on host.

    # ---- node rebalancing permutation ----
    # SPMD needs identical (tile, quarter) cell sizes on all cores, so each
    # cell pays the cross-core max; rebalance the (free) node->core mapping
    # to equalize cells. Rank nodes by in-degree; rank r -> position r//8
    # (fixing each node's source quarter), then greedily pick the core per
    # rank-group of 8 to balance per-quarter in-degree sums within the tile.
    deg_tot = np.bincount(col, minlength=N)
    rank_nodes = np.argsort(-deg_tot, kind="stable")   # node id per rank
    pos_of = np.empty(N, np.int64)
    pos_of[rank_nodes] = np.arange(N) // 8             # position within core
    q_of = np.minimum(pos_of // QT, 3)                 # source quarter
    d2 = np.bincount(col * 4 + q_of[row], minlength=4 * N).reshape(N, 4)
    core_of = np.empty(N, np.int64)
    R = np.zeros((NC, 4), np.int64)
    for m in range(N // 8):
        if m % 128 == 0:
            R[:] = 0                                   # new tile
        grp = rank_nodes[8 * m:8 * m + 8]
        avail = list(range(NC))
        for n in grp:
            cand = R[avail] + d2[n]
            k = avail[int(np.argmin(cand.max(axis=1) * 1000
                                    + cand.sum(axis=1)))]
            core_of[n] = k
            R[k] += d2[n]
            avail.remove(k)
    newid = core_of * SH + pos_of
    orig_of_new = np.empty(N, np.int64)
    orig_of_new[newid] = np.arange(N)
    row = newid[row]
    col = newid[col]
    batch = batch[orig_of_new]

    # host-side degree / dinv (symmetric GCN norm incl. self loop)
    deg = np.bincount(col, minlength=N).astype(np.float64) + 1.0
    dinv = (1.0 / np.sqrt(deg)).astype(np.float32)          # [N]

    kd = col // SH                    # destination core
    ld = col - kd * SH                # local dst row
    ks = row // SH                    # source core
    rr = row - ks * SH
    jq = rr // QT                     # source quarter (0..3)
    idx16 = (ks * QT + (rr - jq * QT)).astype(np.int64)
    tile = ld // 128
    stile = tile // 4

    # per-core sorted streams: (supertile, j, tile, ld)
    per_core = []
    for k in range(NC):
        m = kd == k
        o = np.lexsort((ld[m], tile[m], jq[m], stile[m]))
        per_core.append({
            "tile": tile[m][o], "j": jq[m][o],
            "idx16": idx16[m][o], "ld": ld[m][o],
        })

    # static cell sizes: cell = (tile, j); cross-core max (SPMD: identical
    # window list on all cores); packed tight (no 16-rounding).
    ncell = NT * 4
    S = np.zeros(ncell, dtype=np.int64)
    for k in range(NC):
        ck = per_core[k]["tile"] * 4 + per_core[k]["j"]
        cnt = np.bincount(ck, minlength=ncell)
        S = np.maximum(S, cnt)
    # every (tile, j) cell gets >=1 slot so every (st, j) PSUM bracket and
    # every epilogue exists (pad slots carry scal=-1000 -> zero one-hot)
    S = np.maximum(S, 1)

    # slot layout: groups (st, j) padded to 128-multiples; cells within a
    # group contiguous, tight-packed.
    cell_off = np.zeros(ncell, dtype=np.int64)   # slot offset by cell id
    off = 0
    for st in range(NST):
        for j in range(4):
            for a in range(4):
                c = (4 * st + a) * 4 + j
                cell_off[c] = off
                off += S[c]
            off = _ceil_to(off, 128)   # pad group end to 128
    TOT = off                          # total slots per core

    # group (st, j) sizes/offsets for gathers; g16 = 16-granular gather
    # count (slot space stays 128-padded for window alignment; slots in
    # [g16, gpad) are never gathered -> stale tokens, zeroed by scal=-1000)
    groups = []      # (st, j, slot_off, slots, padded_slots, gather_cnt)
    for st in range(NST):
        for j in range(4):
            c0 = (4 * st) * 4 + j
            goff = cell_off[c0]
            gsz = int(sum(S[(4 * st + a) * 4 + j] for a in range(4)))
            gpad = _ceil_to(gsz, 128)
            g16 = _ceil_to(gsz, 16)
            groups.append((st, j, int(goff), gsz, gpad, g16))

    # fill per-core padded streams
    idx_slots = np.zeros((NC, TOT), dtype=np.int16)
    scal_slots = np.full((NC, TOT), -1000.0, dtype=np.float32)
    for k in range(NC):
        pk = per_core[k]
        ck = pk["tile"] * 4 + pk["j"]
        arange = np.arange(len(ck))
        if len(ck):
            starts_pos = np.concatenate(
                [[0], np.flatnonzero(np.diff(ck) != 0) + 1])
            first_occ = np.zeros(ncell, dtype=np.int64)
            first_occ[ck[starts_pos]] = starts_pos
            within = arange - first_occ[ck]
        else:
            within = arange
        slot = cell_off[ck] + within
        idx_slots[k, slot] = pk["idx16"].astype(np.int16)
        scal_slots[k, slot] = pk["ld"].astype(np.float32)

    # wrapped idx layout per gather group: [16, S/16] tiled to [128, S/16]
    IDXCOLS = TOT // 16
    idx_stream = np.zeros((NC, 128, IDXCOLS), dtype=np.int16)
    gcol_off = {}
    coff = 0
    for (st, j, goff, gsz, gpad, g16) in groups:
        gcol_off[(st, j)] = coff
        if g16 == 0:
            continue
        blk = idx_slots[:, goff:goff + g16].reshape(NC, g16 // 16, 16)
        blk = np.transpose(blk, (0, 2, 1))        # [NC, 16, S/16]
        idx_stream[:, :, coff:coff + g16 // 16] = np.tile(blk, (1, 8, 1))
        coff += g16 // 16

    # windows: one scatter-matmul per (128-slot chunk x intersecting cell).
    ld_slots = scal_slots
    win_cols = []
    chunks = []
    for (st, j, goff, gsz, gpad, g16) in groups:
        for a in range(4):
            c = (4 * st + a) * 4 + j
            if S[c] == 0:
                continue
            c0, c1 = cell_off[c], cell_off[c] + int(S[c])
            ch_lo, ch_hi = c0 // 128, (c1 - 1) // 128
            for ci in range(ch_lo, ch_hi + 1):
                slot0 = ci * 128
                colv = ld_slots[:, slot0:slot0 + 128] - 128.0 * (4 * st + a)
                win_cols.append(colv.astype(np.float32))
                chunks.append(dict(
                    st=st, j=j, a=a,
                    tok_col=int(slot0 - goff) // 128,
                    scal_col=len(win_cols) - 1,
                    base=a * 128,
                ))
    NWIN = len(win_cols)
    scal_stream = np.stack(win_cols, axis=2)  # [NC, 128, NWIN]

    # pooling: batch scalars per core per node tile [128, NT]
    batch = np.asarray(batch, dtype=np.int64)
    batch_scal = np.full((NC, 128, NT), -1000.0, dtype=np.float32)
    for k in range(NC):
        bs = batch[k * SH:(k + 1) * SH].astype(np.float32)
        pad = np.full(SHP - SH, -1000.0, dtype=np.float32)
        bp = np.concatenate([bs, pad]).reshape(NT, 128).T
        batch_scal[k] = bp

    # per-graph 1/count (host): replicated [128, 512]
    cnt = np.bincount(batch, minlength=G).astype(np.float32)
    recip = (1.0 / np.maximum(cnt, 1.0)).astype(np.float32)
    recip_rep = np.tile(recip[None, :], (128, 1))            # [128, G]

    # per-core dinv replicated [128, SHP] fp16
    dinv_rep = np.zeros((NC, 128, SHP), dtype=FP16)
    for k in range(NC):
        dv = np.zeros(SHP, dtype=np.float32)
        dv[:SH] = dinv[k * SH:(k + 1) * SH]
        dinv_rep[k] = np.tile(dv[None, :].astype(FP16), (128, 1))

    # AG-in DMA segments per supertile: (tile_a0, ntiles, j, rowoff)
    ag_segs = []
    for st in range(NST):
        segs = []
        a = 0
        while a < 4:
            base = 512 * st + 128 * a
            j = base // QT
            r = base - j * QT
            n = 1
            while a + n < 4 and (base + 128 * n) // QT == j:
                n += 1
            segs.append((a, n, j, r))
            a += n
        ag_segs.append(segs)

    meta = dict(
        N=N, D=D, SH=SH, SHP=SHP, QT=QT, NT=NT, NST=NST, TBL=TBL, G=G,
        TOT=TOT, NWIN=NWIN, IDXCOLS=IDXCOLS,
        groups=groups, gcol_off=gcol_off, chunks=chunks, ag_segs=ag_segs,
        idx_stream=idx_stream, scal_stream=scal_stream,
        batch_scal=batch_scal, dinv_rep=dinv_rep, recip_rep=recip_rep,
        perm=orig_of_new,
    )
    return meta


def _build(meta):
    """Construct the Bass module (SPMD; identical program on 8 cores)."""
    import concourse.mybir as mybir
    import concourse.bacc as bacc
    import concourse.tile as tile

    f32 = mybir.dt.float32
    fp16 = mybir.dt.float16
    i16 = mybir.dt.int16

    SHP, QT, NT, NST, TBL, G = (meta["SHP"], meta["QT"], meta["NT"],
                                meta["NST"], meta["TBL"], meta["G"])
    NWIN, IDXCOLS = meta["NWIN"], meta["IDXCOLS"]
    groups, gcol_off, chunks, ag_segs = (meta["groups"], meta["gcol_off"],
                                         meta["chunks"], meta["ag_segs"])
    MAXGCOL = max((g[4] // 128 for g in groups), default=1)

    nc = bacc.Bacc("TRN2", target_bir_lowering=False, debug=False,
                   enable_asserts=False, num_devices=NC)

    # ---- I/O ----
    xT_in = nc.dram_tensor("xT_in", [128, SHP], f32, kind="ExternalInput")
    idx_in = nc.dram_tensor("idx_in", [128, IDXCOLS], i16, kind="ExternalInput")
    scal_in = nc.dram_tensor("scal_in", [128, NWIN], f32, kind="ExternalInput")
    bscal_in = nc.dram_tensor("bscal_in", [128, NT], f32, kind="ExternalInput")
    dinv_in = nc.dram_tensor("dinv_in", [128, SHP], fp16, kind="ExternalInput")
    recip_in = nc.dram_tensor("recip_in", [128, G], f32, kind="ExternalInput")
    w_in = nc.dram_tensor("w_in", [5 * 128, 128], fp16, kind="ExternalInput")
    ball_in = nc.dram_tensor("ball_in", [128, 5], f32, kind="ExternalInput")
    iota128_in = nc.dram_tensor("iota128_in", [128, 128], fp16, kind="ExternalInput")
    iotag_in = nc.dram_tensor("iotag_in", [128, G], fp16, kind="ExternalInput")
    ident_in = nc.dram_tensor("ident_in", [128, 128], fp16, kind="ExternalInput")
    wl1_in = nc.dram_tensor("wl1_in", [640, 640], f32, kind="ExternalInput")
    bl1_in = nc.dram_tensor("bl1_in", [128, 5], f32, kind="ExternalInput")
    wl2_in = nc.dram_tensor("wl2_in", [128, 5], f32, kind="ExternalInput")
    bl2_in = nc.dram_tensor("bl2_in", [1, 1], f32, kind="ExternalInput")
    out_ext = nc.dram_tensor("out", [G], f32, kind="ExternalOutput")

    # ---- internal DRAM (collectives) ----
    ag_ins, ag_outs = [], []
    for l in range(5):
        ag_ins.append([nc.dram_tensor(f"agi_{l}_{j}", [QT, 128], fp16,
                                      kind="Internal") for j in range(4)])
        ag_outs.append([nc.dram_tensor(f"ago_{l}_{j}", [TBL, 128], fp16,
                                       kind="Internal", addr_space="Shared")
                        for j in range(4)])
    ar_in = nc.dram_tensor("ar_in", [640, 512], f32, kind="Internal")
    ar_out = nc.dram_tensor("ar_out", [640, 512], f32, kind="Internal",
                            addr_space="Shared")

    AOT = mybir.AluOpType
    AFT = mybir.ActivationFunctionType

    ch_by_stj = {}
    for ch in chunks:
        ch_by_stj.setdefault((ch["st"], ch["j"]), []).append(ch)
    # S>=1 forcing guarantees every (st, j) bracket and epilogue exists
    assert all(ch_by_stj.get((st, j)) for st in range(NST) for j in range(4))

    with tile.TileContext(nc) as tc:
        with tc.tile_pool(name="const", bufs=1) as cpool, \
             tc.tile_pool(name="stream", bufs=1) as spool, \
             tc.tile_pool(name="big", bufs=1) as bpool, \
             tc.tile_pool(name="work", bufs=2) as wpool, \
             tc.tile_pool(name="tokp", bufs=2) as tokpool, \
             tc.tile_pool(name="idxp", bufs=2) as ipool, \
             tc.tile_pool(name="mp", bufs=4) as mpool, \
             tc.tile_pool(name="psA", bufs=2, space="PSUM") as psA, \
             tc.tile_pool(name="psB", bufs=2, space="PSUM") as psB, \
             tc.tile_pool(name="psP", bufs=1, space="PSUM") as psP:

            # ---- constants ----
            iota128 = cpool.tile([128, 128], fp16)
            nc.sync.dma_start(iota128[:], iota128_in.ap())
            iotag = cpool.tile([128, G], fp16)
            nc.sync.dma_start(iotag[:], iotag_in.ap())
            ident = cpool.tile([128, 128], fp16)
            nc.sync.dma_start(ident[:], ident_in.ap())
            w_sb = cpool.tile([128, 5, 128], fp16)
            nc.sync.dma_start(w_sb[:], w_in.ap().rearrange("(a p) b -> p a b", p=128))
            ball = cpool.tile([128, 5], f32)
            nc.sync.dma_start(ball[:], ball_in.ap())
            bscal = cpool.tile([128, NT], f32)
            nc.sync.dma_start(bscal[:], bscal_in.ap())
            dinv_rep = bpool.tile([128, SHP], fp16)
            nc.sync.dma_start(dinv_rep[:], dinv_in.ap())

            scal_sb = spool.tile([128, NWIN], f32)
            nc.sync.dma_start(scal_sb[:], scal_in.ap())

            # y ping-pong buffers (feat-major, fp16)
            yT = [bpool.tile([128, SHP], fp16, name=f"yT{i}", tag=f"yT{i}")
                  for i in range(2)]
            nc.gpsimd.dma_start(yT[0][:], xT_in.ap())   # cast f32->fp16

            # SBUF accumulator for chunk partial sums (chunk 0 copies over
            # whatever is there, chunks 1-3 add)
            acc = bpool.tile([128, SHP], fp16, name="acc", tag="acc")

            def gemm_st(l, st):
                """GEMM + dinv-scale + transpose + AG-input DMAs for one
                supertile of layer l; leaves ut=dinv*u in ycur's slice."""
                ycur_l = yT[l % 2]
                s0 = 512 * st
                ups = psA.tile([128, 512], f32, tag="mm")
                nc.tensor.matmul(ups[:], w_sb[:, l, :],
                                 ycur_l[:, s0:s0 + 512], start=True, stop=True)
                ut = ycur_l[:, s0:s0 + 512]   # reuse consumed input buffer
                nc.vector.tensor_tensor(ut, ups[:],
                                        dinv_rep[:, s0:s0 + 512], AOT.mult)
                trp = psB.tile([128, 512], fp16, tag="tr")
                for a in range(4):
                    nc.tensor.transpose(trp[:, 128 * a:128 * a + 128],
                                        ut[:, 128 * a:128 * a + 128], ident[:])
                agst = wpool.tile([128, 4, 128], fp16, tag="agst")
                nc.vector.tensor_copy(
                    agst[:].rearrange("p a b -> p (a b)"), trp[:])
                for (a0, ntil, j, roff) in ag_segs[st]:
                    nc.sync.dma_start(
                        ag_ins[l][j].ap()[roff:roff + 128 * ntil, :]
                        .rearrange("(a p) b -> p a b", p=128),
                        agst[:, a0:a0 + ntil, :])

            def fire_ag(l, jj):
                nc.gpsimd.collective_compute(
                    "AllGather", AOT.bypass,
                    replica_groups=[list(range(NC))],
                    ins=[ag_ins[l][jj].ap().opt()],
                    outs=[ag_outs[l][jj].ap().opt()])

            # AG chunk jj is complete once this supertile's gemm is done
            ag_bound = {((jj + 1) * QT - 1) // 512: jj for jj in range(4)}

            pool_ps = None
            # layer 0 prologue: gemm + AGs (later layers interleave into the
            # previous layer's epilogue loop)
            scope_g = nc.named_scope("L0_gemm")
            scope_g.__enter__()
            for st in range(NST):
                gemm_st(0, st)
                if st in ag_bound:
                    fire_ag(0, ag_bound[st])
            scope_g.__exit__(None, None, None)

            for l in range(5):
                ycur, ynext = yT[l % 2], yT[(l + 1) % 2]
                # ---- SpMM, chunk-outer (j, then st) ----
                scope_s = nc.named_scope(f"L{l}_spmm")
                scope_s.__enter__()
                for j in range(4):
                    for st in range(NST):
                        cl = ch_by_stj.get((st, j))
                        if not cl:
                            continue
                        gidx = 4 * st + j
                        (gst, gj, goff, gsz, gpad, g16) = groups[gidx]
                        assert gst == st and gj == j
                        if g16 == 0:
                            continue
                        tok = tokpool.tile([128, MAXGCOL, 128], fp16, tag="tok")
                        co = gcol_off[(st, j)]
                        idxt = ipool.tile([128, 8 * MAXGCOL], i16, tag="idx")
                        nc.sync.dma_start(idxt[:, :g16 // 16],
                                          idx_in.ap()[:, co:co + g16 // 16])
                        nc.gpsimd.dma_gather(
                            tok[:, :gpad // 128, :], ag_outs[l][j].ap(),
                            idxt[:, :g16 // 16],
                            num_idxs=g16, num_idxs_reg=g16, elem_size=128,
                            single_packet=False,
                        )
                        zps = psA.tile([128, 512], f32, tag="mm")
                        s0 = 512 * st
                        for i, ch in enumerate(cl):
                            m = mpool.tile([128, 128], fp16, tag="M")
                            d0 = s0 + ch["base"]
                            nc.vector.scalar_tensor_tensor(
                                m[:], iota128[:],
                                scal_sb[:, ch["scal_col"]:ch["scal_col"] + 1],
                                dinv_rep[:, d0:d0 + 128],
                                AOT.is_equal, AOT.mult)
                            nc.tensor.matmul(
                                zps[:, ch["base"]:ch["base"] + 128],
                                tok[:, ch["tok_col"], :], m[:],
                                start=(i == 0), stop=(i == len(cl) - 1))
                        # accumulate chunk partial into SBUF acc
                        if j == 0:
                            nc.vector.tensor_copy(acc[:, s0:s0 + 512], zps[:])
                        else:
                            nc.vector.tensor_tensor(acc[:, s0:s0 + 512],
                                                    zps[:],
                                                    acc[:, s0:s0 + 512],
                                                    AOT.add)
                        if j < 3:
                            continue
                        # ---- epilogue (after last chunk of this st) ----
                        # y = relu(z + dinv^2*u + b); ut = dinv*u is in ycur
                        selft = wpool.tile([128, 512], f32, tag="selft")
                        nc.vector.tensor_tensor(selft[:], ycur[:, s0:s0 + 512],
                                                dinv_rep[:, s0:s0 + 512],
                                                AOT.mult)
                        nc.vector.tensor_tensor(selft[:], acc[:, s0:s0 + 512],
                                                selft[:], AOT.add)
                        nc.scalar.activation(ynext[:, s0:s0 + 512], selft[:],
                                             AFT.Relu, bias=ball[:, l:l + 1])
                        # ---- pooling (node->graph one-hot matmuls) ----
                        trp2 = psB.tile([128, 512], fp16, tag="tr")
                        for a in range(4):
                            nc.tensor.transpose(
                                trp2[:, 128 * a:128 * a + 128],
                                ynext[:, s0 + 128 * a:s0 + 128 * (a + 1)],
                                ident[:])
                        ynm = wpool.tile([128, 4, 128], fp16, tag="ynm")
                        nc.vector.tensor_copy(
                            ynm[:].rearrange("p a b -> p (a b)"), trp2[:])
                        if st == 0:
                            pool_ps = psP.tile([128, 512], f32, tag="pool")
                        for a in range(4):
                            t = 4 * st + a
                            mp = mpool.tile([128, G], fp16, tag="Mp")
                            nc.vector.tensor_scalar(
                                mp[:], iotag[:], bscal[:, t:t + 1], None,
                                AOT.is_equal)
                            nc.tensor.matmul(
                                pool_ps[:, :G], ynm[:, a, :], mp[:],
                                start=(st == 0 and a == 0),
                                stop=(st == NST - 1 and a == 3))
                        # ---- next layer's GEMM for this supertile + AG ----
                        if l < 4:
                            gemm_st(l + 1, st)
                            if st in ag_bound:
                                fire_ag(l + 1, ag_bound[st])
                scope_s.__exit__(None, None, None)
                arst = wpool.tile([128, 512], f32, tag="arst")
                nc.vector.tensor_copy(arst[:, :G], pool_ps[:, :G])
                if G < 512:
                    nc.vector.memset(arst[:, G:], 0.0)
                nc.sync.dma_start(ar_in.ap()[128 * l:128 * (l + 1), :], arst[:])

            nc.gpsimd.collective_compute(
                "AllReduce", AOT.add, replica_groups=[list(range(NC))],
                ins=[ar_in.ap().opt()], outs=[ar_out.ap().opt()])

            # ---- MLP (replicated, fp32) ----
            wl1 = bpool.tile([128, 5, 640], f32)
            nc.sync.dma_start(wl1[:],
                              wl1_in.ap().rearrange("(a p) b -> p a b", p=128))
            wl2 = cpool.tile([128, 5], f32)
            nc.sync.dma_start(wl2[:], wl2_in.ap())
            bl1 = cpool.tile([128, 5], f32)
            nc.sync.dma_start(bl1[:], bl1_in.ap())
            bl2 = cpool.tile([1, 1], f32)
            nc.sync.dma_start(bl2[:], bl2_in.ap())
            rrep = cpool.tile([128, G], f32)
            nc.sync.dma_start(rrep[:], recip_in.ap())

            pm = [wpool.tile([128, 512], f32, tag=f"pm{t}", bufs=1,
                             name=f"pm{t}") for t in range(5)]
            for t in range(5):
                pt = wpool.tile([128, 512], f32, tag="pt")
                nc.sync.dma_start(pt[:], ar_out.ap()[128 * t:128 * (t + 1), :])
                nc.vector.tensor_tensor(pm[t][:, :G], pt[:, :G], rrep[:],
                                        AOT.mult)
                if G < 512:
                    nc.vector.memset(pm[t][:, G:], 0.0)
            yps = psP.tile([1, 512], f32, tag="yf")
            for o in range(5):
                hps = psA.tile([128, 512], f32, tag="mm")
                for i in range(5):
                    nc.tensor.matmul(hps[:], wl1[:, i, 128 * o:128 * (o + 1)],
                                     pm[i][:], start=(i == 0), stop=(i == 4))
                hso = wpool.tile([128, 512], f32, tag="hs")
                nc.scalar.activation(hso[:], hps[:], AFT.Relu,
                                     bias=bl1[:, o:o + 1])
                nc.tensor.matmul(yps[:], wl2[:, o:o + 1], hso[:],
                                 start=(o == 0), stop=(o == 4))
            ysb = wpool.tile([1, 512], f32, tag="ysb")
            nc.scalar.activation(ysb[:], yps[:], AFT.Identity, bias=bl2[:, 0:1])
            nc.sync.dma_start(out_ext.ap().rearrange("(a b) -> a b", a=1),
                              ysb[:, :G])

    nc.compile()
    return nc


def _make_in_maps(meta, x, W_list, b_list, Wl1, bl1, Wl2, bl2):
    N, D, SH, SHP, NT, G = (meta["N"], meta["D"], meta["SH"], meta["SHP"],
                            meta["NT"], meta["G"])
    iota128 = np.tile(np.arange(128), (128, 1)).astype(FP16)
    iotag = np.tile(np.arange(G), (128, 1)).astype(FP16)
    ident = np.eye(128).astype(FP16)
    w_stack = np.concatenate([w.astype(FP16) for w in W_list], axis=0)  # [640,128]
    ball = np.stack([b.astype(np.float32) for b in b_list], axis=1)     # [128,5]
    bl1m = np.asarray(bl1, np.float32).reshape(5, 128).T                # [128,5]
    wl2m = np.asarray(Wl2, np.float32).reshape(5, 128).T                # [128,5]
    wl1m = np.asarray(Wl1, np.float32)
    bl2m = np.asarray(bl2, np.float32).reshape(1, 1)

    xp = np.asarray(x, np.float32)[meta["perm"]]
    in_maps = []
    for k in range(NC):
        xs = xp[k * SH:(k + 1) * SH]
        xT = np.zeros((128, SHP), np.float32)
        xT[:, :SH] = xs.T
        in_maps.append(dict(
            xT_in=xT, idx_in=meta["idx_stream"][k],
            scal_in=meta["scal_stream"][k], bscal_in=meta["batch_scal"][k],
            dinv_in=meta["dinv_rep"][k], recip_in=meta["recip_rep"],
            w_in=w_stack, ball_in=ball, iota128_in=iota128, iotag_in=iotag,
            ident_in=ident,
            wl1_in=wl1m, bl1_in=bl1m, wl2_in=wl2m, bl2_in=bl2m,
        ))
    return in_maps


_LAST_RESULT = {}


def kernel(x, edge_index, batch, W1, b1, W2, b2, W3, b3, W4, b4,
           Wl1, bl1, Wl2, bl2, n_graphs=_G_DEFAULT, trace=False):
    from concourse import bass_utils

    x = np.asarray(x)
    meta = _preprocess(x, np.asarray(edge_index), np.asarray(batch), n_graphs)
    nc = _build(meta)
    in_maps = _make_in_maps(meta, x, [W1, W2, W3, W4, W4],
                            [b1, b2, b3, b4, b4], Wl1, bl1, Wl2, bl2)
    res = bass_utils.run_bass_kernel_spmd(
        nc, in_maps, core_ids=list(range(NC)), trace=trace)
    _LAST_RESULT["res"] = res
    return res.results[0]["out"].astype(np.float32)
